# revision 35
# baseline (speedup 1.0000x reference)
"""WaveNet residual block (dilated causal conv + gated activation + 1x1s)
on 8 Trainium2 NeuronCores, data-parallel over the batch dimension.

Per core (one batch element):
  pre  = W_conv * x (K=3, dil=4, causal)  + W_cond @ cond        (2R=256 rows)
  z    = tanh(preA + bA) * sigmoid(preB + bB)                    (R=128 rows)
  skip = W_skip @ z + b_skip ; out = W_out @ z + b_out

Matmuls run on the PE with the channel dim as the contraction: the dilated
conv is 3 shifted [128x128]x[128,T] matmuls accumulated in PSUM together
with the conditioning 1x1.  The conv/cond stage runs in bf16 (its rounding
error is damped by the saturating tanh/sigmoid that follow); the
output-facing skip/out 1x1s run in float32r (full fp32 via 2-pass PE).
"""

import sys

if "/opt/trn_rl_repo" not in sys.path:
    sys.path.insert(0, "/opt/trn_rl_repo")

import numpy as np

R = 128          # residual channels
S = 256          # skip channels
CIN = 128
CCOND = 80
KW = 3
DIL = 4
PAD = (KW - 1) * DIL   # 8, causal left pad
B = 8
T = 16000
TCH = 500        # time-chunk per PSUM bank (<=512 fp32)
NIO = 2          # compute chunks per IO (DMA) chunk
TIO = TCH * NIO
NCH = T // TIO   # 16 IO chunks

# conv/cond-stage matmul dtype: fp16 halves the input DMA bytes at ~5e-4
# rounding (vs fp32r's exact fp32); the rounding is further damped by the
# saturating tanh/sigmoid that follow.  Post (skip/out) 1x1s stay float32r.
CONV_DT = "float16"   # "float32r" | "float16" | "bfloat16"
POST_DT = "float32r"
STORE_DT = "float32"  # dtype of out/skip in DRAM ("float32" | "float16");
                      # fp16 halves store DMA, host upconverts after gather

_CACHE = {}
LAST_RESULTS = None


def _build_nc():
    import concourse.bacc as bacc
    import concourse.bass as bass
    import concourse.mybir as mybir
    import concourse.tile as tile

    f32 = mybir.dt.float32
    cdt = getattr(mybir.dt, CONV_DT)
    pdt = getattr(mybir.dt, POST_DT)
    sdt = getattr(mybir.dt, STORE_DT)
    AF = mybir.ActivationFunctionType

    nc = bacc.Bacc("TRN2", target_bir_lowering=False, debug=False,
                   enable_asserts=False)

    x_d = nc.dram_tensor("x", (CIN, T + PAD), cdt, kind="ExternalInput")
    c_d = nc.dram_tensor("cond", (CCOND, T), cdt, kind="ExternalInput")
    # [cin, half*3+tap, out128]
    wconv_d = nc.dram_tensor("wconv", (CIN, 6, 128), cdt, kind="ExternalInput")
    # [ccond, half, out128]
    wcond_d = nc.dram_tensor("wcond", (CCOND, 2, 128), cdt, kind="ExternalInput")
    # [r, skipA|skipB|out] transposed 1x1 weights
    wpost_d = nc.dram_tensor("wpost", (R, 384), pdt, kind="ExternalInput")
    # columns: b_convA, b_convB, b_skipA, b_skipB, b_out
    bias_d = nc.dram_tensor("biases", (128, 5), f32, kind="ExternalInput")

    out_d = nc.dram_tensor("out", (R, T), sdt, kind="ExternalOutput")
    skip_d = nc.dram_tensor("skip", (S, T), sdt, kind="ExternalOutput")

    skip_ap = skip_d.ap().rearrange("(h p) t -> p h t", h=2)

    with tile.TileContext(nc) as tc:
        with (
            tc.tile_pool(name="wp", bufs=1) as wp,
            tc.tile_pool(name="ld", bufs=5) as ld,
            tc.tile_pool(name="io", bufs=4) as io,
            tc.tile_pool(name="zp", bufs=4) as zp,
            tc.tile_pool(name="ppre", bufs=2, space=bass.MemorySpace.PSUM) as ppre,
            tc.tile_pool(name="ppost", bufs=1, space=bass.MemorySpace.PSUM) as ppost,
        ):
            # weight loads off the sync queue so the first x/cond loads go out
            # immediately
            wconv = wp.tile([CIN, 6, 128], cdt, name="wconv_sb")
            nc.scalar.dma_start(wconv[:], wconv_d[:])
            wcond = wp.tile([CCOND, 2, 128], cdt, name="wcond_sb")
            nc.scalar.dma_start(wcond[:], wcond_d[:])
            wpost = wp.tile([R, 384], pdt, name="wpost_sb")
            nc.gpsimd.dma_start(wpost[:], wpost_d[:])
            biases = wp.tile([128, 5], f32, name="bias_sb")
            nc.gpsimd.dma_start(biases[:], bias_d[:])

            for ci in range(NCH):
                ti = ci * TIO
                xt = ld.tile([CIN, TIO + PAD], cdt, name="xt", tag="xt")
                nc.sync.dma_start(xt[:], x_d[:, ti:ti + TIO + PAD])
                ct = ld.tile([CCOND, TIO], cdt, name="ct", tag="ct")
                nc.sync.dma_start(ct[:], c_d[:, ti:ti + TIO])
                sk = io.tile([128, 2, TIO], sdt, name="sk", tag="sk")
                ot = io.tile([128, TIO], sdt, name="ot", tag="ot")

                for cj in range(NIO):
                    u0 = cj * TCH   # offset within IO tile
                    preA = ppre.tile([128, TCH], f32, name="preA", tag="preA")
                    preB = ppre.tile([128, TCH], f32, name="preB", tag="preB")
                    for h, pre in ((0, preA), (1, preB)):
                        for k in range(KW):
                            nc.tensor.matmul(
                                pre[:],
                                wconv[:, 3 * h + k, :],
                                xt[:, u0 + k * DIL:u0 + k * DIL + TCH],
                                start=(k == 0), stop=False,
                            )
                        nc.tensor.matmul(
                            pre[:],
                            wcond[:, h, :],
                            ct[:, u0:u0 + TCH],
                            start=False, stop=True,
                        )

                    zA = zp.tile([128, TCH], f32, name="zA", tag="zA")
                    nc.scalar.activation(zA[:], preA[:], AF.Tanh,
                                         bias=biases[:, 0:1])
                    zB = zp.tile([128, TCH], f32, name="zB", tag="zB")
                    nc.scalar.activation(zB[:], preB[:], AF.Sigmoid,
                                         bias=biases[:, 1:2])
                    z = zp.tile([128, TCH], pdt, name="z", tag="z")
                    nc.vector.tensor_mul(z[:], zA[:], zB[:])

                    sA = ppost.tile([128, TCH], f32, name="sA", tag="sA")
                    nc.tensor.matmul(sA[:], wpost[:, 0:128], z[:],
                                     start=True, stop=True)
                    sB = ppost.tile([128, TCH], f32, name="sB", tag="sB")
                    nc.tensor.matmul(sB[:], wpost[:, 128:256], z[:],
                                     start=True, stop=True)
                    op = ppost.tile([128, TCH], f32, name="op", tag="op",
                                    bufs=2)
                    nc.tensor.matmul(op[:], wpost[:, 256:384], z[:],
                                     start=True, stop=True)

                    # psum -> sbuf with bias; out goes via ACT (same table as
                    # tanh/sigmoid), skip halves via DVE
                    nc.scalar.activation(ot[:, u0:u0 + TCH], op[:],
                                         AF.Identity, bias=biases[:, 4:5])
                    nc.vector.tensor_scalar_add(sk[:, 0, u0:u0 + TCH], sA[:],
                                                biases[:, 2:3])
                    nc.vector.tensor_scalar_add(sk[:, 1, u0:u0 + TCH], sB[:],
                                                biases[:, 3:4])

                    if ci >= NCH - 2:
                        # tail: store per compute-chunk, skip half 0 through
                        # the by-now-idle ACT queue instead of the backlogged
                        # SWDGE queue, so the final flush overlaps compute
                        t1 = ti + u0
                        nc.sync.dma_start(out_d[:, t1:t1 + TCH],
                                          ot[:, u0:u0 + TCH])
                        nc.scalar.dma_start(skip_ap[:, 0, t1:t1 + TCH],
                                            sk[:, 0, u0:u0 + TCH])
                        nc.sync.dma_start(skip_ap[:, 1, t1:t1 + TCH],
                                          sk[:, 1, u0:u0 + TCH])

                if ci < NCH - 2:
                    nc.sync.dma_start(out_d[:, ti:ti + TIO], ot[:])
                    nc.gpsimd.dma_start(skip_ap[:, 0, ti:ti + TIO], sk[:, 0, :])
                    nc.sync.dma_start(skip_ap[:, 1, ti:ti + TIO], sk[:, 1, :])

    nc.compile()
    return nc


def _get_nc():
    if "nc" not in _CACHE:
        _CACHE["nc"] = _build_nc()
    return _CACHE["nc"]


def _prep_host(input, cond_input, weight_conv, bias_conv, weight_cond,
               weight_out, bias_out, weight_skip, bias_skip):
    import ml_dtypes
    f = np.float32
    _np_dt = {"float32r": np.float32, "float16": np.float16,
              "bfloat16": ml_dtypes.bfloat16}
    cnp = _np_dt[CONV_DT]
    pnp = _np_dt[POST_DT]

    input = np.asarray(input, f)
    cond_input = np.asarray(cond_input, f)
    weight_conv = np.asarray(weight_conv, f)
    bias_conv = np.asarray(bias_conv, f)
    weight_cond = np.asarray(weight_cond, f)
    weight_out = np.asarray(weight_out, f)
    bias_out = np.asarray(bias_out, f)
    weight_skip = np.asarray(weight_skip, f)
    bias_skip = np.asarray(bias_skip, f)

    x_pad = np.pad(input, ((0, 0), (0, 0), (PAD, 0))).astype(cnp)
    cond_c = cond_input.astype(cnp)

    wconv = np.empty((CIN, 6, 128), f)
    for h in range(2):
        for k in range(KW):
            wconv[:, 3 * h + k, :] = weight_conv[h * 128:(h + 1) * 128, :, k].T
    wcond = np.empty((CCOND, 2, 128), f)
    for h in range(2):
        wcond[:, h, :] = weight_cond[h * 128:(h + 1) * 128, :, 0].T
    wpost = np.empty((R, 384), f)
    wpost[:, 0:128] = weight_skip[0:128, :, 0].T
    wpost[:, 128:256] = weight_skip[128:256, :, 0].T
    wpost[:, 256:384] = weight_out[:, :, 0].T
    biases = np.empty((128, 5), f)
    biases[:, 0] = bias_conv[0:128]
    biases[:, 1] = bias_conv[128:256]
    biases[:, 2] = bias_skip[0:128]
    biases[:, 3] = bias_skip[128:256]
    biases[:, 4] = bias_out

    wconv = np.ascontiguousarray(wconv.astype(cnp))
    wcond = np.ascontiguousarray(wcond.astype(cnp))
    wpost = np.ascontiguousarray(wpost.astype(pnp))
    biases = np.ascontiguousarray(biases)

    in_maps = []
    for b in range(B):
        in_maps.append({
            "x": np.ascontiguousarray(x_pad[b]),
            "cond": np.ascontiguousarray(cond_c[b]),
            "wconv": wconv,
            "wcond": wcond,
            "wpost": wpost,
            "biases": biases,
        })
    return in_maps


def kernel(**inputs):
    global LAST_RESULTS
    from concourse import bass_utils

    nc = _get_nc()
    in_maps = _prep_host(**inputs)
    res = bass_utils.run_bass_kernel_spmd(nc, in_maps, core_ids=list(range(B)))
    LAST_RESULTS = res
    out = np.stack([res.results[b]["out"] for b in range(B)]).astype(np.float32)
    skip = np.stack([res.results[b]["skip"] for b in range(B)]).astype(np.float32)
    return (out, skip)


# revision 36
# speedup vs baseline: 1.0868x; 1.0868x over previous
"""WaveNet residual block (dilated causal conv + gated activation + 1x1s)
on 8 Trainium2 NeuronCores, data-parallel over the batch dimension.

Per core (one batch element):
  pre  = W_conv * x (K=3, dil=4, causal)  + W_cond @ cond        (2R=256 rows)
  z    = tanh(preA + bA) * sigmoid(preB + bB)                    (R=128 rows)
  skip = W_skip @ z + b_skip ; out = W_out @ z + b_out

Matmuls run on the PE with the channel dim as the contraction: the dilated
conv is 3 shifted [128x128]x[128,T] matmuls accumulated in PSUM together
with the conditioning 1x1.  The conv/cond stage runs in bf16 (its rounding
error is damped by the saturating tanh/sigmoid that follow); the
output-facing skip/out 1x1s run in float32r (full fp32 via 2-pass PE).
"""

import sys

if "/opt/trn_rl_repo" not in sys.path:
    sys.path.insert(0, "/opt/trn_rl_repo")

import numpy as np

R = 128          # residual channels
S = 256          # skip channels
CIN = 128
CCOND = 80
KW = 3
DIL = 4
PAD = (KW - 1) * DIL   # 8, causal left pad
B = 8
T = 16000
TCH = 500        # time-chunk per PSUM bank (<=512 fp32)
NIO = 2          # compute chunks per IO (DMA) chunk
TIO = TCH * NIO
NCH = T // TIO   # 16 IO chunks

# conv/cond-stage matmul dtype: fp16 halves the input DMA bytes at ~5e-4
# rounding (vs fp32r's exact fp32); the rounding is further damped by the
# saturating tanh/sigmoid that follow.  Post (skip/out) 1x1s stay float32r.
CONV_DT = "float16"   # "float32r" | "float16" | "bfloat16"
POST_DT = "float32r"
STORE_DT = "float32"  # dtype of out/skip in DRAM ("float32" | "float16");
                      # fp16 halves store DMA, host upconverts after gather

_CACHE = {}
LAST_RESULTS = None


def _build_nc():
    import concourse.bacc as bacc
    import concourse.bass as bass
    import concourse.mybir as mybir
    import concourse.tile as tile

    f32 = mybir.dt.float32
    cdt = getattr(mybir.dt, CONV_DT)
    pdt = getattr(mybir.dt, POST_DT)
    sdt = getattr(mybir.dt, STORE_DT)
    AF = mybir.ActivationFunctionType

    nc = bacc.Bacc("TRN2", target_bir_lowering=False, debug=False,
                   enable_asserts=False)

    x_d = nc.dram_tensor("x", (CIN, T + PAD), cdt, kind="ExternalInput")
    c_d = nc.dram_tensor("cond", (CCOND, T), cdt, kind="ExternalInput")
    # [cin, half*3+tap, out128]
    wconv_d = nc.dram_tensor("wconv", (CIN, 6, 128), cdt, kind="ExternalInput")
    # [ccond, half, out128]
    wcond_d = nc.dram_tensor("wcond", (CCOND, 2, 128), cdt, kind="ExternalInput")
    # [r, skipA|skipB|out] transposed 1x1 weights
    wpost_d = nc.dram_tensor("wpost", (R, 384), pdt, kind="ExternalInput")
    # columns: b_convA, b_convB, b_skipA, b_skipB, b_out
    bias_d = nc.dram_tensor("biases", (128, 5), f32, kind="ExternalInput")

    out_d = nc.dram_tensor("out", (R, T), sdt, kind="ExternalOutput")
    skip_d = nc.dram_tensor("skip", (S, T), sdt, kind="ExternalOutput")

    skip_ap = skip_d.ap().rearrange("(h p) t -> p h t", h=2)

    with tile.TileContext(nc) as tc:
        with (
            tc.tile_pool(name="wp", bufs=1) as wp,
            tc.tile_pool(name="ld", bufs=5) as ld,
            tc.tile_pool(name="io", bufs=4) as io,
            tc.tile_pool(name="zp", bufs=4) as zp,
            tc.tile_pool(name="ppre", bufs=2, space=bass.MemorySpace.PSUM) as ppre,
            tc.tile_pool(name="ppost", bufs=1, space=bass.MemorySpace.PSUM) as ppost,
        ):
            # weight loads off the sync queue so the first x/cond loads go out
            # immediately
            wconv = wp.tile([CIN, 6, 128], cdt, name="wconv_sb")
            nc.scalar.dma_start(wconv[:], wconv_d[:])
            wcond = wp.tile([CCOND, 2, 128], cdt, name="wcond_sb")
            nc.scalar.dma_start(wcond[:], wcond_d[:])
            wpost = wp.tile([R, 384], pdt, name="wpost_sb")
            nc.gpsimd.dma_start(wpost[:], wpost_d[:])
            biases = wp.tile([128, 5], f32, name="bias_sb")
            nc.gpsimd.dma_start(biases[:], bias_d[:])

            for ci in range(NCH):
                ti = ci * TIO
                xt = ld.tile([CIN, TIO + PAD], cdt, name="xt", tag="xt")
                nc.sync.dma_start(xt[:], x_d[:, ti:ti + TIO + PAD])
                ct = ld.tile([CCOND, TIO], cdt, name="ct", tag="ct")
                nc.sync.dma_start(ct[:], c_d[:, ti:ti + TIO])
                sk = io.tile([128, 2, TIO], sdt, name="sk", tag="sk")
                ot = io.tile([128, TIO], sdt, name="ot", tag="ot")

                for cj in range(NIO):
                    u0 = cj * TCH   # offset within IO tile
                    preA = ppre.tile([128, TCH], f32, name="preA", tag="preA")
                    preB = ppre.tile([128, TCH], f32, name="preB", tag="preB")
                    for h, pre in ((0, preA), (1, preB)):
                        for k in range(KW):
                            nc.tensor.matmul(
                                pre[:],
                                wconv[:, 3 * h + k, :],
                                xt[:, u0 + k * DIL:u0 + k * DIL + TCH],
                                start=(k == 0), stop=False,
                            )
                        nc.tensor.matmul(
                            pre[:],
                            wcond[:, h, :],
                            ct[:, u0:u0 + TCH],
                            start=False, stop=True,
                        )

                    zA = zp.tile([128, TCH], f32, name="zA", tag="zA")
                    nc.scalar.activation(zA[:], preA[:], AF.Tanh,
                                         bias=biases[:, 0:1])
                    zB = zp.tile([128, TCH], f32, name="zB", tag="zB")
                    nc.scalar.activation(zB[:], preB[:], AF.Sigmoid,
                                         bias=biases[:, 1:2])
                    z = zp.tile([128, TCH], pdt, name="z", tag="z")
                    nc.vector.tensor_mul(z[:], zA[:], zB[:])

                    sA = ppost.tile([128, TCH], f32, name="sA", tag="sA")
                    nc.tensor.matmul(sA[:], wpost[:, 0:128], z[:],
                                     start=True, stop=True)
                    sB = ppost.tile([128, TCH], f32, name="sB", tag="sB")
                    nc.tensor.matmul(sB[:], wpost[:, 128:256], z[:],
                                     start=True, stop=True)
                    op = ppost.tile([128, TCH], f32, name="op", tag="op",
                                    bufs=2)
                    nc.tensor.matmul(op[:], wpost[:, 256:384], z[:],
                                     start=True, stop=True)

                    # psum -> sbuf with bias; out goes via ACT (same table as
                    # tanh/sigmoid), skip halves via DVE
                    nc.scalar.activation(ot[:, u0:u0 + TCH], op[:],
                                         AF.Identity, bias=biases[:, 4:5])
                    nc.vector.tensor_scalar_add(sk[:, 0, u0:u0 + TCH], sA[:],
                                                biases[:, 2:3])
                    nc.vector.tensor_scalar_add(sk[:, 1, u0:u0 + TCH], sB[:],
                                                biases[:, 3:4])

                nc.sync.dma_start(out_d[:, ti:ti + TIO], ot[:])
                if ci >= NCH - 2:
                    # ACT queue is done computing by now — final skip halves
                    # flush through it instead of the backlogged SWDGE queue
                    nc.scalar.dma_start(skip_ap[:, 0, ti:ti + TIO], sk[:, 0, :])
                else:
                    nc.gpsimd.dma_start(skip_ap[:, 0, ti:ti + TIO], sk[:, 0, :])
                nc.sync.dma_start(skip_ap[:, 1, ti:ti + TIO], sk[:, 1, :])

    nc.compile()
    return nc


def _get_nc():
    if "nc" not in _CACHE:
        _CACHE["nc"] = _build_nc()
    return _CACHE["nc"]


def _prep_host(input, cond_input, weight_conv, bias_conv, weight_cond,
               weight_out, bias_out, weight_skip, bias_skip):
    import ml_dtypes
    f = np.float32
    _np_dt = {"float32r": np.float32, "float16": np.float16,
              "bfloat16": ml_dtypes.bfloat16}
    cnp = _np_dt[CONV_DT]
    pnp = _np_dt[POST_DT]

    input = np.asarray(input, f)
    cond_input = np.asarray(cond_input, f)
    weight_conv = np.asarray(weight_conv, f)
    bias_conv = np.asarray(bias_conv, f)
    weight_cond = np.asarray(weight_cond, f)
    weight_out = np.asarray(weight_out, f)
    bias_out = np.asarray(bias_out, f)
    weight_skip = np.asarray(weight_skip, f)
    bias_skip = np.asarray(bias_skip, f)

    x_pad = np.pad(input, ((0, 0), (0, 0), (PAD, 0))).astype(cnp)
    cond_c = cond_input.astype(cnp)

    wconv = np.empty((CIN, 6, 128), f)
    for h in range(2):
        for k in range(KW):
            wconv[:, 3 * h + k, :] = weight_conv[h * 128:(h + 1) * 128, :, k].T
    wcond = np.empty((CCOND, 2, 128), f)
    for h in range(2):
        wcond[:, h, :] = weight_cond[h * 128:(h + 1) * 128, :, 0].T
    wpost = np.empty((R, 384), f)
    wpost[:, 0:128] = weight_skip[0:128, :, 0].T
    wpost[:, 128:256] = weight_skip[128:256, :, 0].T
    wpost[:, 256:384] = weight_out[:, :, 0].T
    biases = np.empty((128, 5), f)
    biases[:, 0] = bias_conv[0:128]
    biases[:, 1] = bias_conv[128:256]
    biases[:, 2] = bias_skip[0:128]
    biases[:, 3] = bias_skip[128:256]
    biases[:, 4] = bias_out

    wconv = np.ascontiguousarray(wconv.astype(cnp))
    wcond = np.ascontiguousarray(wcond.astype(cnp))
    wpost = np.ascontiguousarray(wpost.astype(pnp))
    biases = np.ascontiguousarray(biases)

    in_maps = []
    for b in range(B):
        in_maps.append({
            "x": np.ascontiguousarray(x_pad[b]),
            "cond": np.ascontiguousarray(cond_c[b]),
            "wconv": wconv,
            "wcond": wcond,
            "wpost": wpost,
            "biases": biases,
        })
    return in_maps


def kernel(**inputs):
    global LAST_RESULTS
    from concourse import bass_utils

    nc = _get_nc()
    in_maps = _prep_host(**inputs)
    res = bass_utils.run_bass_kernel_spmd(nc, in_maps, core_ids=list(range(B)))
    LAST_RESULTS = res
    out = np.stack([res.results[b]["out"] for b in range(B)]).astype(np.float32)
    skip = np.stack([res.results[b]["skip"] for b in range(B)]).astype(np.float32)
    return (out, skip)
